# revision 39
# baseline (speedup 1.0000x reference)
# CapsuleNetwork Trainium2 kernel (8-core data parallel, 4 images/core).
#
# Per core:
#   PE warm-up burst (dummy matmuls) lifts the HAM clock gate to 2.4GHz
#   conv1 3->256 k9 s1 in fp8 DoubleRow: the whole K=243 contraction fits
#     one [128 x 2-slot] DR instruction; x sent as 27 kw-shifted fp8 rows
#     per image and shift-replicated on-chip into the 243-row im2col
#   conv2 256->256 k9 s2 in fp8 DoubleRow, (ig0,ig1) as the DR pair
#     (K=256/instr).  Weights are single-level fp8 with 2D-serpentine
#     error-feedback quantization (noise shaping): neighboring-tap weight
#     rounding errors anti-correlate and cancel against the spatially
#     smooth activations (rel_fro 0.005 vs 0.031 plain-rounded).
#     (og, tap)-outer / (image, y-half)-inner: one weight load serves 8
#     matmuls; 8 PSUM banks accumulate; w2 streamed in tap order.
#   capsule squash (pixel-major), dynamic routing (3 iters) without
#   materializing u_hat:
#     F'[(r,i),(r',c)] = sum_p capsW[p,(r,i)] * exp(b)[p,(r',c)]; G = diag blocks
#     s[c,o]  = sum_{r,i} G[r,c,i] * route_w[r,c,i,o]   (per-class matmuls)
#     b     += caps @ T_block,  T_block[(r,i),(r,c)] = sum_o w[r,c,i,o] v[c,o]
import functools
from contextlib import ExitStack

import numpy as np
import ml_dtypes

import concourse.bass as bass
import concourse.tile as tile
from concourse import bacc
from concourse import mybir
from concourse.bass_utils import run_bass_kernel_spmd

BF = mybir.dt.bfloat16
FP8 = mybir.dt.float8e4
F32 = mybir.dt.float32
AF = mybir.ActivationFunctionType
AX = mybir.AxisListType
DR = mybir.MatmulPerfMode.DoubleRow
W1SCALE = 16.0     # fp8 conv1 weight pre-scale; undone at PSUM drain
W2SCALE = 128.0    # fp8 conv2 weight pre-scale; undone at PSUM drain

NCORES = 8
B = 4              # images per core
PIX = 576          # 24*24 conv2 output pixels
PIX_CHUNKS = [(0, 128), (128, 128), (256, 128), (384, 128), (512, 64)]
R, D, C, O = 32, 8, 10, 16


def _build_nc():
    nc = bacc.Bacc("TRN2", target_bir_lowering=False, debug=False)
    # host-built im2col in DoubleRow layout: [b, ki, slot, 4096], row
    # L = kh*27 + c*9 + kw at (ki=L%128, slot=L//128), pad rows zero
    x_d = nc.declare_dram_parameter("x", [B, 128, 2, 4096], FP8, isOutput=False)
    w1_d = nc.declare_dram_parameter("w1", [128, 2, 256], FP8, isOutput=False)
    b1_d = nc.declare_dram_parameter("b1", [256, 1], F32, isOutput=False)
    # conv2 weights fp8 (noise-shaped), [og, kh, ki, kw*ig*mo]
    w2_d = nc.declare_dram_parameter("w2", [2, 9, 128, 9 * 2 * 128], FP8, isOutput=False)
    b2_d = nc.declare_dram_parameter("b2", [256, 1], F32, isOutput=False)
    ws_d = nc.declare_dram_parameter("ws", [256, C * O], BF, isOutput=False)
    wcob_d = nc.declare_dram_parameter("wcob", [O, C, 256], BF, isOutput=False)
    maskg_d = nc.declare_dram_parameter("maskg", [2, 128, R * C], F32, isOutput=False)
    idf_d = nc.declare_dram_parameter("idf", [128, 128], F32, isOutput=False)
    idb_d = nc.declare_dram_parameter("idb", [128, 128], BF, isOutput=False)
    vout_d = nc.declare_dram_parameter("v_out", [B * C, O], F32, isOutput=True)

    with tile.TileContext(nc) as tc, ExitStack() as ctx:
        consts = ctx.enter_context(tc.tile_pool(name="consts", bufs=1))
        w1t = consts.tile([128, 2, 256], FP8, tag="w1t", name="w1t")
        nc.sync.dma_start(w1t, w1_d[:, :, :])
        b1t = [consts.tile([128, 1], F32, tag=f"b1_{m}", name=f"b1_{m}") for m in range(2)]
        b2t = [consts.tile([128, 1], F32, tag=f"b2_{m}", name=f"b2_{m}") for m in range(2)]
        for m in range(2):
            nc.scalar.dma_start(b1t[m], b1_d[m * 128:(m + 1) * 128, :])
            nc.scalar.dma_start(b2t[m], b2_d[m * 128:(m + 1) * 128, :])
        ws_t = [consts.tile([128, C * O], BF, tag=f"ws{m}", name=f"ws{m}") for m in range(2)]
        for m in range(2):
            nc.scalar.dma_start(ws_t[m], ws_d[m * 128:(m + 1) * 128, :])
        wcob = consts.tile([O, C, 256], BF, tag="wcob", name="wcob")
        nc.scalar.dma_start(wcob, wcob_d[:, :, :])
        idf = consts.tile([128, 128], F32, tag="idf", name="idf")
        idb = consts.tile([128, 128], BF, tag="idb", name="idb")
        nc.sync.dma_start(idf, idf_d[:, :])
        nc.sync.dma_start(idb, idb_d[:, :])
        # block-diag masks: maskg[m][j, r*C+c] = (r == m*16 + j//8)
        maskg = [consts.tile([128, R * C], F32, tag=f"mg{m}", name=f"mg{m}")
                 for m in range(2)]
        for m in range(2):
            nc.scalar.dma_start(maskg[m], maskg_d[m])
        ones16 = consts.tile([16, 1], F32, tag="ones16", name="ones16")
        ones1 = consts.tile([1, 16], F32, tag="ones1", name="ones1")
        nc.vector.memset(ones16, 1.0)
        nc.vector.memset(ones1, 1.0)
        eps_t = consts.tile([128, 1], F32, tag="eps", name="eps")
        nc.gpsimd.memset(eps_t, 1e-8)

        # ---- persistent caps tensors (written during conv phase) ----
        persist = ctx.enter_context(tc.tile_pool(name="persist", bufs=1))
        caps_bf = [persist.tile([128, B, 256], BF, tag=f"cbf{k}", name=f"cbf{k}")
                   for k in range(5)]                              # pixel-major squashed
        capsT = [[persist.tile([128, PIX], BF, tag=f"cT{b}_{g}", name=f"cT{b}_{g}")
                  for g in range(2)] for b in range(B)]            # channel-major squashed
        capsum = [persist.tile([128, B], F32, tag=f"cs{g}", name=f"cs{g}")
                  for g in range(2)]
        capsT_raw = [[persist.tile([128, PIX], BF, tag=f"cr{b}_{g}", name=f"cr{b}_{g}")
                      for g in range(2)] for b in range(B)]

        # ---- PE warm-up: ~5us of dummy matmuls lifts HAM to 2.4GHz ----
        wsrc = persist.tile([128, 512], BF, tag="wsrc", name="wsrc")
        nc.vector.memset(wsrc, 0.0)
        with tc.tile_pool(name="warmps", bufs=1, space="PSUM") as warmps:
            wp = warmps.tile([128, 512], F32, tag="wp", name="wp")
            for _ in range(20):
                nc.tensor.matmul(wp, wsrc[:, 0:128], wsrc, start=True, stop=True)

        rpool = ctx.enter_context(tc.tile_pool(name="rpool", bufs=1))
        dpool = ctx.enter_context(tc.tile_pool(name="dtmp", bufs=6))
        blog = [rpool.tile([128, B, R, C], BF, tag=f"bl{k}", name=f"bl{k}")
                for k in range(5)]

        # ================= conv1 (fp8 DoubleRow, K=243 in one pass) ====
        with tc.tile_pool(name="h1pool", bufs=1) as h1pool:
            # h phase-split along x so conv2 rhs reads are stride-1; fp8 with
            # the two ig halves adjacent in dim1 = the DoubleRow pair dim:
            # [128, ig(2), b, y(56), phase(2), x'(28)]
            h8 = h1pool.tile([128, 2, B, 56, 2, 28], FP8, tag="h8", name="h8")

            with tc.tile_pool(name="xpool", bufs=1) as xpool, \
                 tc.tile_pool(name="c1psum", bufs=6, space="PSUM") as c1psum:
                x243s = []
                for b in range(B):
                    x243 = xpool.tile([128, 2, 64, 64], FP8, tag=f"x243_{b}",
                                      name=f"x243_{b}")
                    nc.gpsimd.dma_start(x243.rearrange("p s a b -> p s (a b)"),
                                        x_d[b])
                    x243s.append(x243)
                for m in range(2):
                    for b in range(B):
                        x243 = x243s[b]
                        for n in range(7):  # 448 pixels = 8 rows of 56
                            ps = c1psum.tile([128, 8, 56], F32, tag="c1ps", name="c1ps")
                            nc.tensor.matmul(
                                ps, w1t[:, :, m * 128:(m + 1) * 128],
                                x243[:, :, 8 * n:8 * n + 8, 0:56],
                                start=True, stop=True, perf_mode=DR)
                            # h8 holds 16*relu(h): psum is 16*conv1 (w1
                            # pre-scale), bias tile is 16*b1; the extra 16
                            # comes out in the conv2 drain (1/2048).  One
                            # strided drain covers both x-phases; alternate
                            # ACT / DVE per chunk.
                            dst = h8[:, m, b, 8 * n:8 * n + 8, :, :]
                            srcp = ps.rearrange("p y (x q) -> p y q x", q=2)
                            if (b + n) % 2 == 0:
                                nc.scalar.activation(dst, srcp, AF.Relu,
                                                     bias=b1t[m], scale=1.0)
                            else:
                                nc.vector.tensor_scalar(
                                    dst, srcp, b1t[m], 0.0,
                                    op0=mybir.AluOpType.add,
                                    op1=mybir.AluOpType.max)

            # ===== conv2 + squash + routing, pipelined over image pairs ====
            # Pair A's squash/routing (DVE/ACT-heavy) overlaps pair B's conv2
            # (PE-heavy).  One weight load serves 4 matmuls; 4 PSUM banks
            # accumulate a pair's (image, y-half) tiles; w2 streamed per og.
            def squash_pair(p):
                with tc.tile_pool(name="tpsum", bufs=2, space="PSUM") as tpsum, \
                     tc.tile_pool(name="tbpsum", bufs=1, space="PSUM") as tbpsum, \
                     tc.tile_pool(name="pmraw", bufs=2) as pmpool, \
                     tc.tile_pool(name="sqtmp", bufs=4) as sqpool:
                    for b in (2 * p, 2 * p + 1):
                        # pixel-major transpose + squash + transpose back; big
                        # elementwise ops alternate DVE / GpSimd per chunk
                        for k, (p0, ln) in enumerate(PIX_CHUNKS):
                            ee = nc.gpsimd if k % 2 == 1 else nc.vector
                            pm = pmpool.tile([128, 256], F32, tag="pm", name="pm")
                            for og in range(2):
                                tp = tpsum.tile([128, 128], BF, tag="tp", name="tp")
                                nc.tensor.transpose(tp[:ln, :],
                                                    capsT_raw[b][og][:, p0:p0 + ln],
                                                    idb)
                                if k % 2 == 1:
                                    nc.scalar.activation(
                                        pm[:ln, og * 128:(og + 1) * 128], tp[:ln, :],
                                        AF.Identity)
                                else:
                                    nc.vector.tensor_copy(
                                        pm[:ln, og * 128:(og + 1) * 128], tp[:ln, :])
                            pm3 = pm.rearrange("p (r i) -> p r i", i=D)
                            sq = sqpool.tile([128, R, D], F32, tag="sq", name="sq")
                            nc.scalar.activation(sq[:ln], pm3[:ln], AF.Square)
                            nsq = sqpool.tile([128, R], F32, tag="nsq", name="nsq")
                            nc.vector.reduce_sum(nsq[:ln], sq[:ln], axis=AX.X)
                            a = sqpool.tile([128, R], F32, tag="sqa", name="sqa")
                            nc.scalar.activation(a[:ln], nsq[:ln], AF.Sqrt, bias=eps_t[:ln])
                            nc.vector.scalar_tensor_tensor(
                                a[:ln], nsq[:ln], 1.0, a[:ln],
                                op0=mybir.AluOpType.add, op1=mybir.AluOpType.mult)
                            nc.vector.reciprocal(a[:ln], a[:ln])
                            nc.vector.tensor_mul(a[:ln], nsq[:ln], a[:ln])
                            cbf3 = caps_bf[k][:, b].rearrange("p (r i) -> p r i", i=D)
                            ee.tensor_mul(
                                cbf3[:ln], pm3[:ln],
                                a[:ln].unsqueeze(2).broadcast_to([ln, R, D]))
                            for og in range(2):
                                tb = tbpsum.tile([128, 128], BF, tag="tb", name="tb")
                                nc.tensor.transpose(
                                    tb[:, :ln],
                                    caps_bf[k][:ln, b, og * 128:(og + 1) * 128],
                                    idb[:ln, :ln])
                                if k % 2 == 0:
                                    nc.scalar.activation(capsT[b][og][:, p0:p0 + ln],
                                                         tb[:, :ln], AF.Identity)
                                else:
                                    nc.vector.tensor_copy(capsT[b][og][:, p0:p0 + ln],
                                                          tb[:, :ln])
                        for g in range(2):  # iter-0 capsule sums
                            nc.vector.reduce_sum(capsum[g][:, b:b + 1], capsT[b][g],
                                                 axis=AX.X)

            def v_squash(s4T, p, last):
                """s4T: psum [16 (o), 2, C] -> v4T bf16 [16, 2, C] or v_out."""
                with tc.tile_pool(name="vsq", bufs=1, space="PSUM") as vps:
                    s4T_sb = dpool.tile([16, 2, C], F32, tag="s4Tsb", name="s4Tsb")
                    nc.vector.tensor_copy(s4T_sb, s4T)
                    if last:
                        s4 = vps.tile([2 * C, 16], F32, tag="s4", name="s4")
                        nc.tensor.transpose(s4, s4T_sb, idf[:16, :16])
                        sq = dpool.tile([2 * C, 16], F32, tag="vsq", name="vsq")
                        nc.scalar.activation(sq, s4, AF.Square)
                        nsq = dpool.tile([2 * C, 1], F32, tag="vnsq", name="vnsq")
                        nc.vector.reduce_sum(nsq, sq, axis=AX.X)
                        a = dpool.tile([2 * C, 1], F32, tag="va", name="va")
                        nc.scalar.activation(a, nsq, AF.Sqrt, bias=eps_t[:2 * C])
                        nc.vector.scalar_tensor_tensor(
                            a, nsq, 1.0, a,
                            op0=mybir.AluOpType.add, op1=mybir.AluOpType.mult)
                        nc.vector.reciprocal(a, a)
                        nc.vector.tensor_mul(a, nsq, a)
                        vout = rpool.tile([2 * C, 16], F32, tag=f"vout_{p}",
                                          name=f"vout_{p}")
                        nc.vector.tensor_mul(vout, s4, a.broadcast_to([2 * C, 16]))
                        nc.sync.dma_start(vout_d[2 * p * C:(2 * p + 2) * C, :], vout)
                        return None
                    # row-major squash: partition-reduce |s|^2 via ones-matmul,
                    # broadcast the scale back via a K=1 outer-product matmul.
                    sqT = dpool.tile([16, 2 * C], F32, tag="vsqT", name="vsqT")
                    nc.scalar.activation(sqT, s4T.rearrange("p b c -> p (b c)"),
                                         AF.Square)
                    nsqr = vps.tile([1, 2 * C], F32, tag="nsqr", name="nsqr")
                    nc.tensor.matmul(nsqr, ones16, sqT, start=True, stop=True)
                    a = dpool.tile([1, 2 * C], F32, tag="var", name="var")
                    nc.scalar.activation(a, nsqr, AF.Sqrt, bias=eps_t[:1])
                    nc.vector.scalar_tensor_tensor(
                        a, nsqr, 1.0, a,
                        op0=mybir.AluOpType.add, op1=mybir.AluOpType.mult)
                    nc.vector.reciprocal(a, a)
                    sgr = dpool.tile([1, 2 * C], F32, tag="sgr", name="sgr")
                    nc.vector.tensor_mul(sgr, nsqr, a)
                    sgT = vps.tile([16, 2 * C], F32, tag="sgT", name="sgT")
                    nc.tensor.matmul(sgT, ones1, sgr, start=True, stop=True)
                    v4T = rpool.tile([16, 2, C], BF, tag=f"v4T_{p}", name=f"v4T_{p}")
                    nc.vector.tensor_mul(v4T, s4T_sb,
                                         sgT.rearrange("p (b c) -> p b c", c=C))
                    return v4T

            def b_update(v4T, p, it):
                """b_log += caps . T_block  (T = route_w . v, block-diag in r)."""
                T4 = [rpool.tile([128, 2, R * C], BF, tag=f"T4_{m}_{p}",
                                 name=f"T4_{m}_{p}") for m in range(2)]
                with tc.tile_pool(name="t4ps", bufs=1, space="PSUM") as t4ps, \
                     tc.tile_pool(name="dps", bufs=2, space="PSUM") as dps:
                    for m in range(2):
                        t4 = t4ps.tile([128, C, 2], F32, tag="t4", name="t4")
                        for c in range(C):
                            nc.tensor.matmul(t4[:, c, :],
                                             wcob[:, c, m * 128:(m + 1) * 128],
                                             v4T[:, :, c], start=True, stop=True)
                        data = (t4.transpose([0, 2, 1]).unsqueeze(2)
                                .broadcast_to([128, 2, R, C]))
                        mk = (maskg[m].rearrange("p (r c) -> p r c", c=C)
                              .unsqueeze(1).broadcast_to([128, 2, R, C]))
                        nc.vector.tensor_mul(
                            T4[m].rearrange("p b (r c) -> p b r c", c=C), data, mk)
                    for k, (p0, ln) in enumerate(PIX_CHUNKS):
                        for bl in range(2):
                            b = 2 * p + bl
                            off = (k + bl) % 2 == 1
                            dl = dps.tile([128, R, C], F32, tag="dl", name="dl")
                            for kc in range(2):
                                nc.tensor.matmul(dl[:ln], capsT[b][kc][:, p0:p0 + ln],
                                                 T4[kc][:, bl, :],
                                                 start=(kc == 0), stop=(kc == 1))
                            if it == 0:
                                if off:
                                    nc.scalar.activation(blog[k][:ln, b], dl[:ln],
                                                         AF.Identity)
                                else:
                                    nc.vector.tensor_copy(blog[k][:ln, b], dl[:ln])
                            else:
                                # stage PSUM->bf16 (ACT/DVE), then a cheap
                                # bf16 add on the other engine
                                dsb = dpool.tile([128, R, C], BF, tag="dsb",
                                                 name="dsb", bufs=4)
                                if off:
                                    nc.scalar.activation(dsb[:ln], dl[:ln],
                                                         AF.Identity)
                                    nc.gpsimd.tensor_add(blog[k][:ln, b],
                                                         blog[k][:ln, b], dsb[:ln])
                                else:
                                    nc.vector.tensor_copy(dsb[:ln], dl[:ln])
                                    nc.vector.tensor_add(blog[k][:ln, b],
                                                         blog[k][:ln, b], dsb[:ln])

            def softmax_G(p):
                """softmax over c folded into caps; G = diag blocks of cw.T @ e."""
                e = []
                cw = []
                for k, (p0, ln) in enumerate(PIX_CHUNKS):
                    ee = nc.gpsimd if k % 2 == 1 else nc.vector
                    et = rpool.tile([128, 2, R, C], BF, tag=f"e{k}_{p}",
                                    name=f"e{k}_{p}")
                    nc.scalar.activation(et[:ln], blog[k][:ln, 2 * p:2 * p + 2],
                                         AF.Exp)
                    den = dpool.tile([128, 2, R], F32, tag="den", name="den")
                    nc.vector.reduce_sum(den[:ln], et[:ln], axis=AX.X)
                    nc.vector.reciprocal(den[:ln], den[:ln])
                    cwt = rpool.tile([128, 2, R, D], BF, tag=f"cw{k}_{p}",
                                     name=f"cw{k}_{p}")
                    cbf4 = caps_bf[k].rearrange("p b (r i) -> p b r i", i=D)
                    ee.tensor_mul(
                        cwt[:ln], cbf4[:ln, 2 * p:2 * p + 2],
                        den[:ln].unsqueeze(3).broadcast_to([ln, 2, R, D]))
                    e.append(et)
                    cw.append(cwt)
                Gp = [rpool.tile([128, 2, C], BF, tag=f"G{m}_{p}", name=f"G{m}_{p}")
                      for m in range(2)]
                for m in range(2):
                    with tc.tile_pool(name="fps", bufs=1, space="PSUM") as fps:
                        F4 = fps.tile([128, 2, 512], F32, tag="F4", name="F4")
                        for k, (p0, ln) in enumerate(PIX_CHUNKS):
                            for bl in range(2):
                                cwf = cw[k][:, bl].rearrange("p r i -> p (r i)")
                                ef = e[k][:, bl].rearrange("p r c -> p (r c)")
                                nc.tensor.matmul(F4[:, bl, :R * C],
                                                 cwf[:ln, m * 128:(m + 1) * 128],
                                                 ef[:ln],
                                                 start=(k == 0), stop=(k == 4))
                        fm = dpool.tile([128, 2, R * C], BF, tag="fm", name="fm",
                                        bufs=2)
                        mk = maskg[m].unsqueeze(1).broadcast_to([128, 2, R * C])
                        nc.vector.tensor_mul(fm, F4[:, :, :R * C], mk)
                        gf = dpool.tile([128, 2, C], F32, tag="gf", name="gf")
                        nc.vector.reduce_sum(
                            gf, fm.rearrange("p b (r c) -> p b c r", c=C), axis=AX.X)
                        (nc.gpsimd if m == 1 else nc.vector).tensor_copy(Gp[m], gf)
                return Gp

            def s_matmuls(spool, rhs_pair):
                s4T = spool.tile([16, 2, C], F32, tag="s4T", name="s4T")
                for c in range(C):
                    for m in range(2):
                        rhs = rhs_pair[m]
                        if len(rhs.shape) == 3:
                            rhs = rhs[:, :, c]
                        nc.tensor.matmul(s4T[:, :, c],
                                         ws_t[m][:, c * 16:(c + 1) * 16],
                                         rhs, start=(m == 0), stop=(m == 1))
                return s4T

            def pe_keepwarm(nmm):
                # low-priority PE filler: keeps the HAM clock gate open
                # through the DVE/ACT-heavy routing stretches
                for _ in range(nmm):
                    nc.tensor.matmul(warm_ps, wsrc[:, 0:128], wsrc[:, 0:128],
                                     start=True, stop=True)

            def routing_pair(p):
                # ---- iter 0: uniform coupling ----
                csb = [rpool.tile([128, 2], BF, tag=f"csb{g}_{p}",
                                  name=f"csb{g}_{p}") for g in range(2)]
                for g in range(2):
                    nc.vector.tensor_scalar_mul(csb[g], capsum[g][:, 2 * p:2 * p + 2],
                                                1.0 / C)
                with tc.tile_pool(name="sps0", bufs=1, space="PSUM") as sps:
                    s4T = s_matmuls(sps, csb)
                    v4T = v_squash(s4T, p, last=False)
                b_update(v4T, p, it=0)
                # ---- iters 1, 2 ----
                for it in (1, 2):
                    Gp = softmax_G(p)
                    with tc.tile_pool(name=f"sps{it}", bufs=1, space="PSUM") as sps:
                        s4T = s_matmuls(sps, Gp)
                        v4T = v_squash(s4T, p, last=(it == 2))
                    if it == 1:
                        b_update(v4T, p, it=1)

            with tc.tile_pool(name="w2pool", bufs=1) as w2pool, \
                 tc.tile_pool(name="keepps", bufs=1, space="PSUM") as keepps, \
                 tc.tile_pool(name="c2psum", bufs=1, space="PSUM") as c2psum:
                warm_ps = keepps.tile([128, 128], F32, tag="wk", name="wk")
                for p in range(2):
                    for og in range(2):
                        w2c = {}
                        for kh in range(9):
                            w2c[kh] = w2pool.tile(
                                [128, 9, 2, 128], FP8, tag=f"w2_{kh}",
                                name=f"w2_{kh}")
                            nc.gpsimd.dma_start(
                                w2c[kh].rearrange("p t g m -> p (t g m)"),
                                w2_d[og, kh])
                        pss = [[c2psum.tile([128, 288], F32, tag=f"c2ps_{bl}_{y}",
                                            name=f"c2ps_{bl}_{y}")
                                for y in range(2)] for bl in range(2)]
                        for t81 in range(81):
                            kh, kw = t81 // 9, t81 % 9
                            lhsT = w2c[kh][:, kw, :, :]
                            for bl in range(2):
                                b = 2 * p + bl
                                for y in range(2):
                                    rhs = h8[:, :, b,
                                             kh + 24 * y:kh + 24 * y + 24:2,
                                             kw % 2, kw // 2:kw // 2 + 24]
                                    nc.tensor.matmul(
                                        pss[bl][y], lhsT, rhs,
                                        start=(t81 == 0), stop=(t81 == 80),
                                        perf_mode=DR)
                        for bl in range(2):
                            b = 2 * p + bl
                            for y in range(2):
                                if (bl * 2 + y) % 2 == 0:
                                    nc.scalar.activation(
                                        capsT_raw[b][og][:, y * 288:(y + 1) * 288],
                                        pss[bl][y], AF.Identity, bias=b2t[og],
                                        scale=1.0 / (W1SCALE * W2SCALE))
                                else:
                                    nc.vector.tensor_scalar(
                                        capsT_raw[b][og][:, y * 288:(y + 1) * 288],
                                        pss[bl][y], 1.0 / (W1SCALE * W2SCALE),
                                        b2t[og],
                                        op0=mybir.AluOpType.mult,
                                        op1=mybir.AluOpType.add)
                    squash_pair(p)
                    routing_pair(p)

    nc.compile()
    return nc


@functools.lru_cache(maxsize=1)
def _get_nc():
    return _build_nc()


def _ef2d_e4m3(W):
    """2D serpentine error-feedback quantization over the trailing (kh, kw)
    axes: rounding errors anti-correlate along the kernel window, cancelling
    against spatially smooth activations."""
    e4 = ml_dtypes.float8_e4m3
    out = np.zeros_like(W)
    e = np.zeros(W.shape[:-2], np.float32)
    for kh in range(9):
        rng = range(9) if kh % 2 == 0 else range(8, -1, -1)
        for kw in rng:
            t = W[..., kh, kw] + e
            qv = t.astype(e4).astype(np.float32)
            e = t - qv
            out[..., kh, kw] = qv
    return out


def _prep_consts(conv1_w, conv1_b, conv2_w, conv2_b, route_w):
    bf = ml_dtypes.bfloat16
    e4 = ml_dtypes.float8_e4m3
    f32 = np.float32
    # conv1 weights: noise-shaped fp8, row L = kh*27 + c*9 + kw packed into
    # DoubleRow layout [ki = L%128, slot = L//128, mo], pad rows zero
    w1q = _ef2d_e4m3(conv1_w.astype(f32).transpose(1, 0, 2, 3) * W1SCALE)
    # w1q [c, mo, kh, kw] -> rows [kh, c, kw, mo]
    w1rows = np.zeros((256, 256), f32)
    w1rows[0:243] = w1q.transpose(2, 0, 3, 1).reshape(243, 256)
    w1 = w1rows.reshape(2, 128, 256).transpose(1, 0, 2)    # [ki, slot, mo]
    # conv2 weights: noise-shaped fp8, [og, kh, ki, kw, ig, mo]
    w2q = _ef2d_e4m3(conv2_w.astype(f32) * W2SCALE)        # [mo, ci, kh, kw]
    w2 = (w2q.reshape(2, 128, 2, 128, 9, 9)                # [og, mo, ig, ki, kh, kw]
          .transpose(0, 4, 3, 5, 2, 1))                    # [og, kh, ki, kw, ig, mo]
    ws = route_w.astype(f32).transpose(0, 2, 1, 3).reshape(256, C * O)
    wcob = route_w.astype(f32).transpose(3, 1, 0, 2).reshape(O, C, 256)
    maskg = np.zeros((2, 128, R * C), f32)
    for m in range(2):
        for j in range(128):
            r = m * 16 + j // D
            maskg[m, j, r * C:(r + 1) * C] = 1.0
    return {
        "w1": np.ascontiguousarray(w1).astype(e4),
        "b1": np.ascontiguousarray(W1SCALE * conv1_b.astype(f32).reshape(256, 1)),
        "w2": np.ascontiguousarray(w2).reshape(2, 9, 128, 9 * 2 * 128).astype(e4),
        "b2": np.ascontiguousarray(conv2_b.astype(f32).reshape(256, 1)),
        "ws": np.ascontiguousarray(ws).astype(bf),
        "wcob": np.ascontiguousarray(wcob).astype(bf),
        "idf": np.eye(128, dtype=f32),
        "idb": np.eye(128, dtype=f32).astype(bf),
        "maskg": maskg,
    }


def _prep_x(x):
    """x [nb, 3, 64, 64] f32 -> im2col [nb, 128, 2, 4096] e4m3 in DoubleRow
    layout: row L = kh*27 + c*9 + kw holds x[c, y+kh, x'+kw] at y*64+x'."""
    nb = x.shape[0]
    x27 = np.zeros((nb, 27, 4096), np.float32)
    xv = x27.reshape(nb, 3, 9, 64, 64)
    for kw in range(9):
        xv[:, :, kw, :, 0:64 - kw] = x[:, :, :, kw:64]
    x243 = np.zeros((nb, 256, 4096), np.float32)
    for kh in range(9):
        x243[:, kh * 27:(kh + 1) * 27, 0:4096 - 64 * kh] = x27[:, :, 64 * kh:]
    return np.ascontiguousarray(
        x243.reshape(nb, 2, 128, 4096).transpose(0, 2, 1, 3)
    ).astype(ml_dtypes.float8_e4m3)


def _ensure_ntff_hook():
    """The agent image's antenv lacks axon_hooks; shim it so trace=True works."""
    import sys
    import types
    try:
        from antenv import axon_hooks  # noqa: F401
        return
    except ImportError:
        pass
    mod = types.ModuleType("antenv.axon_hooks")
    _h = [None]
    mod.get_axon_ntff_profile_hook = lambda: _h[0]
    mod.set_axon_ntff_profile_hook = lambda h: _h.__setitem__(0, h)
    sys.modules["antenv.axon_hooks"] = mod
    try:
        from trn_agent_boot.trn_boot import _ntff_profile_via_ctypes
        mod.set_axon_ntff_profile_hook(
            _ntff_profile_via_ctypes("/opt/axon/libaxon_pjrt.so"))
    except Exception as e:  # degrade: trace skipped, run still works
        print(f"ntff hook shim failed: {e}")


def run(x, conv1_w, conv1_b, conv2_w, conv2_b, route_w, trace=False, cores=NCORES):
    if trace:
        _ensure_ntff_hook()
    x = np.asarray(x, np.float32)
    nb = x.shape[0]
    consts = _prep_consts(np.asarray(conv1_w), np.asarray(conv1_b),
                          np.asarray(conv2_w), np.asarray(conv2_b),
                          np.asarray(route_w))
    xb = _prep_x(x)
    assert nb == B * cores
    in_maps = []
    for cid in range(cores):
        m = dict(consts)
        m["x"] = np.ascontiguousarray(xb[cid * B:(cid + 1) * B])
        in_maps.append(m)
    res = run_bass_kernel_spmd(_get_nc(), in_maps, list(range(cores)), trace=trace)
    out = np.concatenate([r["v_out"].reshape(B, C, O) for r in res.results], axis=0)
    return out.astype(np.float32), res


def kernel(x, conv1_w, conv1_b, conv2_w, conv2_b, route_w):
    out, _ = run(x, conv1_w, conv1_b, conv2_w, conv2_b, route_w, trace=False)
    return out


# revision 40
# speedup vs baseline: 1.0131x; 1.0131x over previous
# CapsuleNetwork Trainium2 kernel (8-core data parallel, 4 images/core).
#
# Per core:
#   PE warm-up burst (dummy matmuls) lifts the HAM clock gate to 2.4GHz
#   conv1 3->256 k9 s1 in fp8 DoubleRow: the whole K=243 contraction fits
#     one [128 x 2-slot] DR instruction; x sent as 27 kw-shifted fp8 rows
#     per image and shift-replicated on-chip into the 243-row im2col
#   conv2 256->256 k9 s2 in fp8 DoubleRow, (ig0,ig1) as the DR pair
#     (K=256/instr).  Weights are single-level fp8 with 2D-serpentine
#     error-feedback quantization (noise shaping): neighboring-tap weight
#     rounding errors anti-correlate and cancel against the spatially
#     smooth activations (rel_fro 0.005 vs 0.031 plain-rounded).
#     (og, tap)-outer / (image, y-half)-inner: one weight load serves 8
#     matmuls; 8 PSUM banks accumulate; w2 streamed in tap order.
#   capsule squash (pixel-major), dynamic routing (3 iters) without
#   materializing u_hat:
#     F'[(r,i),(r',c)] = sum_p capsW[p,(r,i)] * exp(b)[p,(r',c)]; G = diag blocks
#     s[c,o]  = sum_{r,i} G[r,c,i] * route_w[r,c,i,o]   (per-class matmuls)
#     b     += caps @ T_block,  T_block[(r,i),(r,c)] = sum_o w[r,c,i,o] v[c,o]
import functools
from contextlib import ExitStack

import numpy as np
import ml_dtypes

import concourse.bass as bass
import concourse.tile as tile
from concourse import bacc
from concourse import mybir
from concourse.bass_utils import run_bass_kernel_spmd

BF = mybir.dt.bfloat16
FP8 = mybir.dt.float8e4
F32 = mybir.dt.float32
AF = mybir.ActivationFunctionType
AX = mybir.AxisListType
DR = mybir.MatmulPerfMode.DoubleRow
W1SCALE = 16.0     # fp8 conv1 weight pre-scale; undone at PSUM drain
W2SCALE = 128.0    # fp8 conv2 weight pre-scale; undone at PSUM drain

NCORES = 8
B = 4              # images per core
PIX = 576          # 24*24 conv2 output pixels
PIX_CHUNKS = [(0, 128), (128, 128), (256, 128), (384, 128), (512, 64)]
R, D, C, O = 32, 8, 10, 16


def _build_nc():
    nc = bacc.Bacc("TRN2", target_bir_lowering=False, debug=False)
    # host-built im2col in DoubleRow layout: [b, ki, slot, 4096], row
    # L = kh*27 + c*9 + kw at (ki=L%128, slot=L//128), pad rows zero
    x_d = nc.declare_dram_parameter("x", [B, 128, 2, 4096], FP8, isOutput=False)
    w1_d = nc.declare_dram_parameter("w1", [128, 2, 256], FP8, isOutput=False)
    b1_d = nc.declare_dram_parameter("b1", [256, 1], F32, isOutput=False)
    # conv2 weights fp8 (noise-shaped), [og, kh, ki, kw*ig*mo]
    w2_d = nc.declare_dram_parameter("w2", [2, 9, 128, 9 * 2 * 128], FP8, isOutput=False)
    b2_d = nc.declare_dram_parameter("b2", [256, 1], F32, isOutput=False)
    ws_d = nc.declare_dram_parameter("ws", [256, C * O], BF, isOutput=False)
    wcob_d = nc.declare_dram_parameter("wcob", [O, C, 256], BF, isOutput=False)
    maskg_d = nc.declare_dram_parameter("maskg", [2, 128, R * C], F32, isOutput=False)
    idf_d = nc.declare_dram_parameter("idf", [128, 128], F32, isOutput=False)
    idb_d = nc.declare_dram_parameter("idb", [128, 128], BF, isOutput=False)
    vout_d = nc.declare_dram_parameter("v_out", [B * C, O], F32, isOutput=True)

    with tile.TileContext(nc) as tc, ExitStack() as ctx:
        consts = ctx.enter_context(tc.tile_pool(name="consts", bufs=1))
        w1t = consts.tile([128, 2, 256], FP8, tag="w1t", name="w1t")
        nc.sync.dma_start(w1t, w1_d[:, :, :])
        b1t = [consts.tile([128, 1], F32, tag=f"b1_{m}", name=f"b1_{m}") for m in range(2)]
        b2t = [consts.tile([128, 1], F32, tag=f"b2_{m}", name=f"b2_{m}") for m in range(2)]
        for m in range(2):
            nc.scalar.dma_start(b1t[m], b1_d[m * 128:(m + 1) * 128, :])
            nc.scalar.dma_start(b2t[m], b2_d[m * 128:(m + 1) * 128, :])
        ws_t = [consts.tile([128, C * O], BF, tag=f"ws{m}", name=f"ws{m}") for m in range(2)]
        for m in range(2):
            nc.scalar.dma_start(ws_t[m], ws_d[m * 128:(m + 1) * 128, :])
        wcob = consts.tile([O, C, 256], BF, tag="wcob", name="wcob")
        nc.scalar.dma_start(wcob, wcob_d[:, :, :])
        idf = consts.tile([128, 128], F32, tag="idf", name="idf")
        idb = consts.tile([128, 128], BF, tag="idb", name="idb")
        nc.sync.dma_start(idf, idf_d[:, :])
        nc.sync.dma_start(idb, idb_d[:, :])
        # block-diag masks: maskg[m][j, r*C+c] = (r == m*16 + j//8)
        maskg = [consts.tile([128, R * C], F32, tag=f"mg{m}", name=f"mg{m}")
                 for m in range(2)]
        for m in range(2):
            nc.scalar.dma_start(maskg[m], maskg_d[m])
        ones16 = consts.tile([16, 1], F32, tag="ones16", name="ones16")
        ones1 = consts.tile([1, 16], F32, tag="ones1", name="ones1")
        nc.vector.memset(ones16, 1.0)
        nc.vector.memset(ones1, 1.0)
        eps_t = consts.tile([128, 1], F32, tag="eps", name="eps")
        nc.gpsimd.memset(eps_t, 1e-8)

        # ---- persistent caps tensors (written during conv phase) ----
        persist = ctx.enter_context(tc.tile_pool(name="persist", bufs=1))
        caps_bf = [persist.tile([128, B, 256], BF, tag=f"cbf{k}", name=f"cbf{k}")
                   for k in range(5)]                              # pixel-major squashed
        capsT = [[persist.tile([128, PIX], BF, tag=f"cT{b}_{g}", name=f"cT{b}_{g}")
                  for g in range(2)] for b in range(B)]            # channel-major squashed
        capsum = [persist.tile([128, B], F32, tag=f"cs{g}", name=f"cs{g}")
                  for g in range(2)]
        capsT_raw = [[persist.tile([128, PIX], BF, tag=f"cr{b}_{g}", name=f"cr{b}_{g}")
                      for g in range(2)] for b in range(B)]

        # ---- PE warm-up: ~5us of dummy matmuls lifts HAM to 2.4GHz ----
        wsrc = persist.tile([128, 512], BF, tag="wsrc", name="wsrc")
        nc.vector.memset(wsrc, 0.0)
        with tc.tile_pool(name="warmps", bufs=1, space="PSUM") as warmps:
            wp = warmps.tile([128, 512], F32, tag="wp", name="wp")
            for _ in range(20):
                nc.tensor.matmul(wp, wsrc[:, 0:128], wsrc, start=True, stop=True)

        rpool = ctx.enter_context(tc.tile_pool(name="rpool", bufs=1))
        dpool = ctx.enter_context(tc.tile_pool(name="dtmp", bufs=6))
        blog = [rpool.tile([128, B, R, C], BF, tag=f"bl{k}", name=f"bl{k}")
                for k in range(5)]

        # ================= conv1 (fp8 DoubleRow, K=243 in one pass) ====
        with tc.tile_pool(name="h1pool", bufs=1) as h1pool:
            # h phase-split along x so conv2 rhs reads are stride-1; fp8 with
            # the two ig halves adjacent in dim1 = the DoubleRow pair dim:
            # [128, ig(2), b, y(56), phase(2), x'(28)]
            h8 = h1pool.tile([128, 2, B, 56, 2, 28], FP8, tag="h8", name="h8")

            with tc.tile_pool(name="xpool", bufs=1) as xpool, \
                 tc.tile_pool(name="c1psum", bufs=6, space="PSUM") as c1psum:
                x243s = []
                for b in range(B):
                    x243 = xpool.tile([128, 2, 64, 64], FP8, tag=f"x243_{b}",
                                      name=f"x243_{b}")
                    nc.gpsimd.dma_start(x243.rearrange("p s a b -> p s (a b)"),
                                        x_d[b])
                    x243s.append(x243)
                for m in range(2):
                    for b in range(B):
                        x243 = x243s[b]
                        for n in range(7):  # 448 pixels = 8 rows of 56
                            ps = c1psum.tile([128, 8, 56], F32, tag="c1ps", name="c1ps")
                            nc.tensor.matmul(
                                ps, w1t[:, :, m * 128:(m + 1) * 128],
                                x243[:, :, 8 * n:8 * n + 8, 0:56],
                                start=True, stop=True, perf_mode=DR)
                            # h8 holds 16*relu(h): psum is 16*conv1 (w1
                            # pre-scale), bias tile is 16*b1; the extra 16
                            # comes out in the conv2 drain (1/2048).  One
                            # strided drain covers both x-phases; alternate
                            # ACT / DVE per chunk.
                            dst = h8[:, m, b, 8 * n:8 * n + 8, :, :]
                            srcp = ps.rearrange("p y (x q) -> p y q x", q=2)
                            if (b + n) % 2 == 0:
                                nc.scalar.activation(dst, srcp, AF.Relu,
                                                     bias=b1t[m], scale=1.0)
                            else:
                                nc.vector.tensor_scalar(
                                    dst, srcp, b1t[m], 0.0,
                                    op0=mybir.AluOpType.add,
                                    op1=mybir.AluOpType.max)

            # ===== conv2 + squash + routing, pipelined over image pairs ====
            # Pair A's squash/routing (DVE/ACT-heavy) overlaps pair B's conv2
            # (PE-heavy).  One weight load serves 4 matmuls; 4 PSUM banks
            # accumulate a pair's (image, y-half) tiles; w2 streamed per og.
            def squash_pair(p):
                with tc.tile_pool(name="tpsum", bufs=2, space="PSUM") as tpsum, \
                     tc.tile_pool(name="tbpsum", bufs=1, space="PSUM") as tbpsum, \
                     tc.tile_pool(name="pmraw", bufs=2) as pmpool, \
                     tc.tile_pool(name="sqtmp", bufs=4) as sqpool:
                    for b in (2 * p, 2 * p + 1):
                        # pixel-major transpose + squash + transpose back; big
                        # elementwise ops alternate DVE / GpSimd per chunk
                        for k, (p0, ln) in enumerate(PIX_CHUNKS):
                            ee = nc.gpsimd if k % 2 == 1 else nc.vector
                            pm = pmpool.tile([128, 256], F32, tag="pm", name="pm")
                            for og in range(2):
                                tp = tpsum.tile([128, 128], BF, tag="tp", name="tp")
                                nc.tensor.transpose(tp[:ln, :],
                                                    capsT_raw[b][og][:, p0:p0 + ln],
                                                    idb)
                                if k % 2 == 1:
                                    nc.scalar.activation(
                                        pm[:ln, og * 128:(og + 1) * 128], tp[:ln, :],
                                        AF.Identity)
                                else:
                                    nc.vector.tensor_copy(
                                        pm[:ln, og * 128:(og + 1) * 128], tp[:ln, :])
                            pm3 = pm.rearrange("p (r i) -> p r i", i=D)
                            sq = sqpool.tile([128, R, D], F32, tag="sq", name="sq")
                            nc.scalar.activation(sq[:ln], pm3[:ln], AF.Square)
                            nsq = sqpool.tile([128, R], F32, tag="nsq", name="nsq")
                            nc.vector.reduce_sum(nsq[:ln], sq[:ln], axis=AX.X)
                            a = sqpool.tile([128, R], F32, tag="sqa", name="sqa")
                            nc.scalar.activation(a[:ln], nsq[:ln], AF.Sqrt, bias=eps_t[:ln])
                            nc.vector.scalar_tensor_tensor(
                                a[:ln], nsq[:ln], 1.0, a[:ln],
                                op0=mybir.AluOpType.add, op1=mybir.AluOpType.mult)
                            nc.vector.reciprocal(a[:ln], a[:ln])
                            nc.vector.tensor_mul(a[:ln], nsq[:ln], a[:ln])
                            cbf3 = caps_bf[k][:, b].rearrange("p (r i) -> p r i", i=D)
                            ee.tensor_mul(
                                cbf3[:ln], pm3[:ln],
                                a[:ln].unsqueeze(2).broadcast_to([ln, R, D]))
                            for og in range(2):
                                tb = tbpsum.tile([128, 128], BF, tag="tb", name="tb")
                                nc.tensor.transpose(
                                    tb[:, :ln],
                                    caps_bf[k][:ln, b, og * 128:(og + 1) * 128],
                                    idb[:ln, :ln])
                                if k % 2 == 0:
                                    nc.scalar.activation(capsT[b][og][:, p0:p0 + ln],
                                                         tb[:, :ln], AF.Identity)
                                else:
                                    nc.vector.tensor_copy(capsT[b][og][:, p0:p0 + ln],
                                                          tb[:, :ln])
                        for g in range(2):  # iter-0 capsule sums
                            nc.vector.reduce_sum(capsum[g][:, b:b + 1], capsT[b][g],
                                                 axis=AX.X)

            def v_squash(s4T, p, last):
                """s4T: psum [16 (o), 2, C] -> v4T bf16 [16, 2, C] or v_out."""
                with tc.tile_pool(name="vsq", bufs=1, space="PSUM") as vps:
                    s4T_sb = dpool.tile([16, 2, C], F32, tag="s4Tsb", name="s4Tsb")
                    nc.vector.tensor_copy(s4T_sb, s4T)
                    if last:
                        s4 = vps.tile([2 * C, 16], F32, tag="s4", name="s4")
                        nc.tensor.transpose(s4, s4T_sb, idf[:16, :16])
                        sq = dpool.tile([2 * C, 16], F32, tag="vsq", name="vsq")
                        nc.scalar.activation(sq, s4, AF.Square)
                        nsq = dpool.tile([2 * C, 1], F32, tag="vnsq", name="vnsq")
                        nc.vector.reduce_sum(nsq, sq, axis=AX.X)
                        a = dpool.tile([2 * C, 1], F32, tag="va", name="va")
                        nc.scalar.activation(a, nsq, AF.Sqrt, bias=eps_t[:2 * C])
                        nc.vector.scalar_tensor_tensor(
                            a, nsq, 1.0, a,
                            op0=mybir.AluOpType.add, op1=mybir.AluOpType.mult)
                        nc.vector.reciprocal(a, a)
                        nc.vector.tensor_mul(a, nsq, a)
                        vout = rpool.tile([2 * C, 16], F32, tag=f"vout_{p}",
                                          name=f"vout_{p}")
                        nc.vector.tensor_mul(vout, s4, a.broadcast_to([2 * C, 16]))
                        nc.sync.dma_start(vout_d[2 * p * C:(2 * p + 2) * C, :], vout)
                        return None
                    # row-major squash: partition-reduce |s|^2 via ones-matmul,
                    # broadcast the scale back via a K=1 outer-product matmul.
                    sqT = dpool.tile([16, 2 * C], F32, tag="vsqT", name="vsqT")
                    nc.scalar.activation(sqT, s4T.rearrange("p b c -> p (b c)"),
                                         AF.Square)
                    nsqr = vps.tile([1, 2 * C], F32, tag="nsqr", name="nsqr")
                    nc.tensor.matmul(nsqr, ones16, sqT, start=True, stop=True)
                    a = dpool.tile([1, 2 * C], F32, tag="var", name="var")
                    nc.scalar.activation(a, nsqr, AF.Sqrt, bias=eps_t[:1])
                    nc.vector.scalar_tensor_tensor(
                        a, nsqr, 1.0, a,
                        op0=mybir.AluOpType.add, op1=mybir.AluOpType.mult)
                    nc.vector.reciprocal(a, a)
                    sgr = dpool.tile([1, 2 * C], F32, tag="sgr", name="sgr")
                    nc.vector.tensor_mul(sgr, nsqr, a)
                    sgT = vps.tile([16, 2 * C], F32, tag="sgT", name="sgT")
                    nc.tensor.matmul(sgT, ones1, sgr, start=True, stop=True)
                    v4T = rpool.tile([16, 2, C], BF, tag=f"v4T_{p}", name=f"v4T_{p}")
                    nc.vector.tensor_mul(v4T, s4T_sb,
                                         sgT.rearrange("p (b c) -> p b c", c=C))
                    return v4T

            def b_update(v4T, p, it):
                """b_log += caps . T_block  (T = route_w . v, block-diag in r)."""
                T4 = [rpool.tile([128, 2, R * C], BF, tag=f"T4_{m}_{p}",
                                 name=f"T4_{m}_{p}") for m in range(2)]
                with tc.tile_pool(name="t4ps", bufs=1, space="PSUM") as t4ps, \
                     tc.tile_pool(name="dps", bufs=2, space="PSUM") as dps:
                    for m in range(2):
                        t4 = t4ps.tile([128, C, 2], F32, tag="t4", name="t4")
                        for c in range(C):
                            nc.tensor.matmul(t4[:, c, :],
                                             wcob[:, c, m * 128:(m + 1) * 128],
                                             v4T[:, :, c], start=True, stop=True)
                        data = (t4.transpose([0, 2, 1]).unsqueeze(2)
                                .broadcast_to([128, 2, R, C]))
                        mk = (maskg[m].rearrange("p (r c) -> p r c", c=C)
                              .unsqueeze(1).broadcast_to([128, 2, R, C]))
                        nc.vector.tensor_mul(
                            T4[m].rearrange("p b (r c) -> p b r c", c=C), data, mk)
                    for k, (p0, ln) in enumerate(PIX_CHUNKS):
                        for bl in range(2):
                            b = 2 * p + bl
                            off = (k + bl) % 2 == 1
                            dl = dps.tile([128, R, C], F32, tag="dl", name="dl")
                            for kc in range(2):
                                nc.tensor.matmul(dl[:ln], capsT[b][kc][:, p0:p0 + ln],
                                                 T4[kc][:, bl, :],
                                                 start=(kc == 0), stop=(kc == 1))
                            if it == 0:
                                if off:
                                    nc.scalar.activation(blog[k][:ln, b], dl[:ln],
                                                         AF.Identity)
                                else:
                                    nc.vector.tensor_copy(blog[k][:ln, b], dl[:ln])
                            elif off:
                                # stage PSUM->SBUF on ACT; add runs on GpSimd
                                dsb = dpool.tile([128, R, C], BF, tag="dsb",
                                                 name="dsb", bufs=4)
                                nc.scalar.activation(dsb[:ln], dl[:ln], AF.Identity)
                                nc.gpsimd.tensor_add(blog[k][:ln, b], blog[k][:ln, b],
                                                     dsb[:ln])
                            else:
                                nc.vector.tensor_add(blog[k][:ln, b], blog[k][:ln, b],
                                                     dl[:ln])

            def softmax_G(p):
                """softmax over c folded into caps; G = diag blocks of cw.T @ e."""
                e = []
                cw = []
                for k, (p0, ln) in enumerate(PIX_CHUNKS):
                    ee = nc.gpsimd if k % 2 == 1 else nc.vector
                    et = rpool.tile([128, 2, R, C], BF, tag=f"e{k}_{p}",
                                    name=f"e{k}_{p}")
                    nc.scalar.activation(et[:ln], blog[k][:ln, 2 * p:2 * p + 2],
                                         AF.Exp)
                    den = dpool.tile([128, 2, R], F32, tag="den", name="den")
                    nc.vector.reduce_sum(den[:ln], et[:ln], axis=AX.X)
                    nc.vector.reciprocal(den[:ln], den[:ln])
                    cwt = rpool.tile([128, 2, R, D], BF, tag=f"cw{k}_{p}",
                                     name=f"cw{k}_{p}")
                    cbf4 = caps_bf[k].rearrange("p b (r i) -> p b r i", i=D)
                    ee.tensor_mul(
                        cwt[:ln], cbf4[:ln, 2 * p:2 * p + 2],
                        den[:ln].unsqueeze(3).broadcast_to([ln, 2, R, D]))
                    e.append(et)
                    cw.append(cwt)
                Gp = [rpool.tile([128, 2, C], BF, tag=f"G{m}_{p}", name=f"G{m}_{p}")
                      for m in range(2)]
                for m in range(2):
                    with tc.tile_pool(name="fps", bufs=1, space="PSUM") as fps:
                        F4 = fps.tile([128, 2, 512], F32, tag="F4", name="F4")
                        for k, (p0, ln) in enumerate(PIX_CHUNKS):
                            for bl in range(2):
                                cwf = cw[k][:, bl].rearrange("p r i -> p (r i)")
                                ef = e[k][:, bl].rearrange("p r c -> p (r c)")
                                nc.tensor.matmul(F4[:, bl, :R * C],
                                                 cwf[:ln, m * 128:(m + 1) * 128],
                                                 ef[:ln],
                                                 start=(k == 0), stop=(k == 4))
                        fm = dpool.tile([128, 2, R * C], BF, tag="fm", name="fm",
                                        bufs=2)
                        mk = maskg[m].unsqueeze(1).broadcast_to([128, 2, R * C])
                        nc.vector.tensor_mul(fm, F4[:, :, :R * C], mk)
                        gf = dpool.tile([128, 2, C], F32, tag="gf", name="gf")
                        nc.vector.reduce_sum(
                            gf, fm.rearrange("p b (r c) -> p b c r", c=C), axis=AX.X)
                        (nc.gpsimd if m == 1 else nc.vector).tensor_copy(Gp[m], gf)
                return Gp

            def s_matmuls(spool, rhs_pair):
                s4T = spool.tile([16, 2, C], F32, tag="s4T", name="s4T")
                for c in range(C):
                    for m in range(2):
                        rhs = rhs_pair[m]
                        if len(rhs.shape) == 3:
                            rhs = rhs[:, :, c]
                        nc.tensor.matmul(s4T[:, :, c],
                                         ws_t[m][:, c * 16:(c + 1) * 16],
                                         rhs, start=(m == 0), stop=(m == 1))
                return s4T

            def pe_keepwarm(nmm):
                # low-priority PE filler: keeps the HAM clock gate open
                # through the DVE/ACT-heavy routing stretches
                for _ in range(nmm):
                    nc.tensor.matmul(warm_ps, wsrc[:, 0:128], wsrc[:, 0:128],
                                     start=True, stop=True)

            def routing_pair(p):
                # ---- iter 0: uniform coupling ----
                csb = [rpool.tile([128, 2], BF, tag=f"csb{g}_{p}",
                                  name=f"csb{g}_{p}") for g in range(2)]
                for g in range(2):
                    nc.vector.tensor_scalar_mul(csb[g], capsum[g][:, 2 * p:2 * p + 2],
                                                1.0 / C)
                with tc.tile_pool(name="sps0", bufs=1, space="PSUM") as sps:
                    s4T = s_matmuls(sps, csb)
                    v4T = v_squash(s4T, p, last=False)
                b_update(v4T, p, it=0)
                # ---- iters 1, 2 ----
                for it in (1, 2):
                    Gp = softmax_G(p)
                    with tc.tile_pool(name=f"sps{it}", bufs=1, space="PSUM") as sps:
                        s4T = s_matmuls(sps, Gp)
                        v4T = v_squash(s4T, p, last=(it == 2))
                    if it == 1:
                        b_update(v4T, p, it=1)

            with tc.tile_pool(name="w2pool", bufs=1) as w2pool, \
                 tc.tile_pool(name="keepps", bufs=1, space="PSUM") as keepps, \
                 tc.tile_pool(name="c2psum", bufs=1, space="PSUM") as c2psum:
                warm_ps = keepps.tile([128, 128], F32, tag="wk", name="wk")
                for p in range(2):
                    for og in range(2):
                        w2c = {}
                        for kh in range(9):
                            w2c[kh] = w2pool.tile(
                                [128, 9, 2, 128], FP8, tag=f"w2_{kh}",
                                name=f"w2_{kh}")
                            nc.gpsimd.dma_start(
                                w2c[kh].rearrange("p t g m -> p (t g m)"),
                                w2_d[og, kh])
                        pss = [[c2psum.tile([128, 288], F32, tag=f"c2ps_{bl}_{y}",
                                            name=f"c2ps_{bl}_{y}")
                                for y in range(2)] for bl in range(2)]
                        for t81 in range(81):
                            kh, kw = t81 // 9, t81 % 9
                            lhsT = w2c[kh][:, kw, :, :]
                            for bl in range(2):
                                b = 2 * p + bl
                                for y in range(2):
                                    rhs = h8[:, :, b,
                                             kh + 24 * y:kh + 24 * y + 24:2,
                                             kw % 2, kw // 2:kw // 2 + 24]
                                    nc.tensor.matmul(
                                        pss[bl][y], lhsT, rhs,
                                        start=(t81 == 0), stop=(t81 == 80),
                                        perf_mode=DR)
                        for bl in range(2):
                            b = 2 * p + bl
                            for y in range(2):
                                if (bl * 2 + y) % 2 == 0:
                                    nc.scalar.activation(
                                        capsT_raw[b][og][:, y * 288:(y + 1) * 288],
                                        pss[bl][y], AF.Identity, bias=b2t[og],
                                        scale=1.0 / (W1SCALE * W2SCALE))
                                else:
                                    nc.vector.tensor_scalar(
                                        capsT_raw[b][og][:, y * 288:(y + 1) * 288],
                                        pss[bl][y], 1.0 / (W1SCALE * W2SCALE),
                                        b2t[og],
                                        op0=mybir.AluOpType.mult,
                                        op1=mybir.AluOpType.add)
                    squash_pair(p)
                    routing_pair(p)

    nc.compile()
    return nc


@functools.lru_cache(maxsize=1)
def _get_nc():
    return _build_nc()


def _ef2d_e4m3(W):
    """2D serpentine error-feedback quantization over the trailing (kh, kw)
    axes: rounding errors anti-correlate along the kernel window, cancelling
    against spatially smooth activations."""
    e4 = ml_dtypes.float8_e4m3
    out = np.zeros_like(W)
    e = np.zeros(W.shape[:-2], np.float32)
    for kh in range(9):
        rng = range(9) if kh % 2 == 0 else range(8, -1, -1)
        for kw in rng:
            t = W[..., kh, kw] + e
            qv = t.astype(e4).astype(np.float32)
            e = t - qv
            out[..., kh, kw] = qv
    return out


def _prep_consts(conv1_w, conv1_b, conv2_w, conv2_b, route_w):
    bf = ml_dtypes.bfloat16
    e4 = ml_dtypes.float8_e4m3
    f32 = np.float32
    # conv1 weights: noise-shaped fp8, row L = kh*27 + c*9 + kw packed into
    # DoubleRow layout [ki = L%128, slot = L//128, mo], pad rows zero
    w1q = _ef2d_e4m3(conv1_w.astype(f32).transpose(1, 0, 2, 3) * W1SCALE)
    # w1q [c, mo, kh, kw] -> rows [kh, c, kw, mo]
    w1rows = np.zeros((256, 256), f32)
    w1rows[0:243] = w1q.transpose(2, 0, 3, 1).reshape(243, 256)
    w1 = w1rows.reshape(2, 128, 256).transpose(1, 0, 2)    # [ki, slot, mo]
    # conv2 weights: noise-shaped fp8, [og, kh, ki, kw, ig, mo]
    w2q = _ef2d_e4m3(conv2_w.astype(f32) * W2SCALE)        # [mo, ci, kh, kw]
    w2 = (w2q.reshape(2, 128, 2, 128, 9, 9)                # [og, mo, ig, ki, kh, kw]
          .transpose(0, 4, 3, 5, 2, 1))                    # [og, kh, ki, kw, ig, mo]
    ws = route_w.astype(f32).transpose(0, 2, 1, 3).reshape(256, C * O)
    wcob = route_w.astype(f32).transpose(3, 1, 0, 2).reshape(O, C, 256)
    maskg = np.zeros((2, 128, R * C), f32)
    for m in range(2):
        for j in range(128):
            r = m * 16 + j // D
            maskg[m, j, r * C:(r + 1) * C] = 1.0
    return {
        "w1": np.ascontiguousarray(w1).astype(e4),
        "b1": np.ascontiguousarray(W1SCALE * conv1_b.astype(f32).reshape(256, 1)),
        "w2": np.ascontiguousarray(w2).reshape(2, 9, 128, 9 * 2 * 128).astype(e4),
        "b2": np.ascontiguousarray(conv2_b.astype(f32).reshape(256, 1)),
        "ws": np.ascontiguousarray(ws).astype(bf),
        "wcob": np.ascontiguousarray(wcob).astype(bf),
        "idf": np.eye(128, dtype=f32),
        "idb": np.eye(128, dtype=f32).astype(bf),
        "maskg": maskg,
    }


def _prep_x(x):
    """x [nb, 3, 64, 64] f32 -> im2col [nb, 128, 2, 4096] e4m3 in DoubleRow
    layout: row L = kh*27 + c*9 + kw holds x[c, y+kh, x'+kw] at y*64+x'."""
    nb = x.shape[0]
    x27 = np.zeros((nb, 27, 4096), np.float32)
    xv = x27.reshape(nb, 3, 9, 64, 64)
    for kw in range(9):
        xv[:, :, kw, :, 0:64 - kw] = x[:, :, :, kw:64]
    x243 = np.zeros((nb, 256, 4096), np.float32)
    for kh in range(9):
        x243[:, kh * 27:(kh + 1) * 27, 0:4096 - 64 * kh] = x27[:, :, 64 * kh:]
    return np.ascontiguousarray(
        x243.reshape(nb, 2, 128, 4096).transpose(0, 2, 1, 3)
    ).astype(ml_dtypes.float8_e4m3)


def _ensure_ntff_hook():
    """The agent image's antenv lacks axon_hooks; shim it so trace=True works."""
    import sys
    import types
    try:
        from antenv import axon_hooks  # noqa: F401
        return
    except ImportError:
        pass
    mod = types.ModuleType("antenv.axon_hooks")
    _h = [None]
    mod.get_axon_ntff_profile_hook = lambda: _h[0]
    mod.set_axon_ntff_profile_hook = lambda h: _h.__setitem__(0, h)
    sys.modules["antenv.axon_hooks"] = mod
    try:
        from trn_agent_boot.trn_boot import _ntff_profile_via_ctypes
        mod.set_axon_ntff_profile_hook(
            _ntff_profile_via_ctypes("/opt/axon/libaxon_pjrt.so"))
    except Exception as e:  # degrade: trace skipped, run still works
        print(f"ntff hook shim failed: {e}")


def run(x, conv1_w, conv1_b, conv2_w, conv2_b, route_w, trace=False, cores=NCORES):
    if trace:
        _ensure_ntff_hook()
    x = np.asarray(x, np.float32)
    nb = x.shape[0]
    consts = _prep_consts(np.asarray(conv1_w), np.asarray(conv1_b),
                          np.asarray(conv2_w), np.asarray(conv2_b),
                          np.asarray(route_w))
    xb = _prep_x(x)
    assert nb == B * cores
    in_maps = []
    for cid in range(cores):
        m = dict(consts)
        m["x"] = np.ascontiguousarray(xb[cid * B:(cid + 1) * B])
        in_maps.append(m)
    res = run_bass_kernel_spmd(_get_nc(), in_maps, list(range(cores)), trace=trace)
    out = np.concatenate([r["v_out"].reshape(B, C, O) for r in res.results], axis=0)
    return out.astype(np.float32), res


def kernel(x, conv1_w, conv1_b, conv2_w, conv2_b, route_w):
    out, _ = run(x, conv1_w, conv1_b, conv2_w, conv2_b, route_w, trace=False)
    return out
